# revision 19
# baseline (speedup 1.0000x reference)
"""Trainium2 Bass kernel for nn_EnhancedTransformerBlock (dense_transformer).

Sharding: 8 cores; core c owns 512 tokens (rows c*512:(c+1)*512 of the
flattened [4096, 1024] input; batch = c//4). k (feature-major) and v
(token-major, head-interleaved with a ones column per head giving fused
softmax denominators) are AllGathered within each batch's quad of cores.
Everything else is data-parallel over tokens; the unshard happens on host.

Activations are feature-major [128p, kt, T] so matmul contraction (over the
feature dim) maps onto PE partitions. LayerNorm affine params are folded into
the following matmul's weights host-side; the per-token mean/inv-std
correction is applied to matmul *outputs* (a rank-1 update), with stats via
ones-matmuls and GPSIMD partition broadcasts. f32r matmuls carry the
adapter/projection chain (full PE rate at N=512); bf16 carries MoE weights
and attention score/value paths.
"""
import os
import sys

sys.path.insert(0, "/opt/trn_rl_repo")

import numpy as np
import ml_dtypes

import concourse.bass as bass
import concourse.tile as tile
from concourse import bacc, mybir
from contextlib import ExitStack

F32 = mybir.dt.float32
BF16 = mybir.dt.bfloat16
F32R = mybir.dt.float32r
AX = mybir.AluOpType
AF = mybir.ActivationFunctionType

D = 1024
H = 16
HD = 64
NE = 4
ND = 8
AD = 256
B = 2
L = 2048
NCORES = 8
T = 512          # tokens per core
KT = D // 128    # 8 feature k-tiles
P = 128
EPS = 1e-5
SCALES = (1, 2, 4)
GROUPS = [[0, 1, 2, 3], [4, 5, 6, 7]]

_COMPILED = {}


# --------------------------------------------------------------------------
# device program
# --------------------------------------------------------------------------

def _build_program(scalars, debug=False):
    c_comb = scalars["c_comb"]
    thr = scalars["thr"]

    nc = bacc.Bacc("TRN2", target_bir_lowering=False, debug=False,
                   num_devices=NCORES)

    def din(name, shape, dt=F32):
        return nc.dram_tensor(name, shape, dt, kind="ExternalInput")

    def dout(name, shape, dt=F32):
        return nc.dram_tensor(name, shape, dt, kind="ExternalOutput")

    d = {}
    d["xT"] = din("xT", [D, T])
    for nm, shp in [("pw", [P, KT]), ("pb", [P, KT]), ("b1f", [P, KT]),
                    ("ws1", [P, KT]), ("bt2", [P, KT]), ("sw2", [P, KT]),
                    ("sb2", [P, KT]), ("bq", [P, KT]), ("bk", [P, KT]),
                    ("bo", [P, KT]), ("attw", [P, KT]), ("attb", [P, KT]),
                    ("moew", [P, KT]), ("moeb", [P, KT]),
                    ("bt1", [P, 2]), ("wst1", [P, 2]),
                    ("blog", [ND, 1]), ("brf", [NE, 1]), ("wsr", [NE, 1]),
                    ("vb", [1, D]), ("ones_c", [P, 1]), ("ones_rw", [1, P])]:
        d[nm] = din(nm, shp)
    d["w1f"] = din("w1f", [KT, P, D])
    d["wlog"] = din("wlog", [KT, P, ND])
    d["wt1"] = din("wt1", [KT, P, AD])
    d["wt2"] = din("wt2", [2, P, D])
    d["wq"] = din("wq", [KT, P, D])
    d["wk"] = din("wk", [KT, P, D])
    d["wv"] = din("wv", [KT, P, D])
    d["wo"] = din("wo", [KT, P, D])
    d["wr"] = din("wr", [KT, P, NE])
    d["p2"] = din("p2", [P, 64], BF16)
    d["ew1"] = din("ew1", [NE, KT, P, 4 * D], BF16)
    d["eb1"] = din("eb1", [NE, P, 32])
    d["ew2"] = din("ew2", [NE, 32, P, D], BF16)
    d["eb2"] = din("eb2", [NE, P, KT])

    d["outT"] = dout("outT", [D, T])
    d["logT"] = dout("logT", [ND, T])
    if debug:
        for nm in ("dbg_h", "dbg_a", "dbg_attn", "dbg_comb"):
            d[nm] = dout(nm, [D, T])
        d["dbg_rw"] = dout("dbg_rw", [NE, T])
        d["dbg_q"] = dout("dbg_q", [D, T], BF16)
        d["dbg_ex"] = dout("dbg_ex", [P, T], BF16)
        d["dbg_pv"] = dout("dbg_pv", [65, T])
        d["dbg_rec"] = dout("dbg_rec", [1, T])
        d["dbg_k"] = dout("dbg_k", [4 * D, T], BF16)
        d["dbg_v"] = dout("dbg_v", [4 * T, H * 65], BF16)

    d["kg_in"] = nc.dram_tensor("kg_in", [D, T], BF16)
    d["kg_out"] = nc.dram_tensor("kg_out", [4 * D, T], BF16)
    d["vg_in"] = nc.dram_tensor("vg_in", [T, H * 65], BF16)
    d["vg_out"] = nc.dram_tensor("vg_out", [4 * T, H * 65], BF16)

    with tile.TileContext(nc) as tc:
        with ExitStack() as st:
            _emit(nc, tc, st, d, c_comb, thr, debug)

    nc.compile()
    return nc


def _emit(nc, tc, st, d, c_comb, thr, debug):
    # Global pool: long-lived activation slots (tag-shared across phases).
    G = st.enter_context(tc.tile_pool(name="G", bufs=1))
    pst = st.enter_context(tc.tile_pool(name="pst", bufs=2, space="PSUM"))

    def dma(dst, src):
        nc.sync.dma_start(dst, src)

    _nm = [0]

    def uname(pfx):
        _nm[0] += 1
        return f"{pfx}{_nm[0]}"

    def big(tag="ACT16", dt=F32R):
        return G.tile([P, KT, T], dt, tag="ACT16", bufs=3,
                      name=uname("big"))

    def bft():
        return G.tile([P, 4, H * 65], BF16, tag="BF", bufs=2,
                      padded_shape=None)

    ones_r = G.tile([P, 1], F32R, tag="ones_r")
    nc.sync.dma_start(ones_r[:], d["ones_c"].ap().bitcast(F32R))
    ones_row = G.tile([1, P], F32R, tag="ones_row")
    nc.sync.dma_start(ones_row[:], d["ones_rw"].ap().bitcast(F32R))

    def load_vec(name, cols, parts=P):
        t = G.tile([parts, cols], F32, tag=name)
        dma(t[:], d[name].ap())
        return t

    def stats(u, nkt=KT):
        pss = pst.tile([1, T], F32, tag="st", bufs=2)
        psq = pst.tile([1, T], F32, tag="st", bufs=2)
        for k in range(nkt):
            sq = G.tile([P, T], F32R, tag="sq", bufs=2)
            nc.vector.tensor_mul(sq[:], u[:, k], u[:, k])
            nc.tensor.matmul(pss[:], ones_r[:], u[:, k],
                             start=(k == 0), stop=(k == nkt - 1))
            nc.tensor.matmul(psq[:], ones_r[:], sq[:],
                             start=(k == 0), stop=(k == nkt - 1))
        row = lambda: G.tile([1, T], F32, tag="srow", bufs=4,
                             name=uname("srow"))
        m = row()
        nc.vector.tensor_scalar(m[:], pss[:], 1.0 / D, None, AX.mult)
        var = row()
        nc.vector.tensor_mul(var[:], m[:], m[:])
        e2 = row()
        nc.vector.tensor_scalar(e2[:], psq[:], 1.0 / D, None, AX.mult)
        nc.vector.tensor_sub(var[:], e2[:], var[:])
        nc.vector.tensor_scalar(var[:], var[:], EPS, None, AX.add)
        s0 = row()
        nc.scalar.activation(s0[:], var[:], AF.Sqrt)
        y0 = row()
        nc.vector.reciprocal_approx_fast(y0[:], s0[:])
        t1 = row()
        nc.vector.tensor_mul(t1[:], y0[:], y0[:])
        nc.vector.tensor_mul(t1[:], t1[:], var[:])
        nc.vector.tensor_scalar(t1[:], t1[:], -0.5, 1.5, AX.mult, AX.add)
        inv = row()
        nc.vector.tensor_mul(inv[:], y0[:], t1[:])
        negmi = row()
        nc.vector.scalar_tensor_tensor(negmi[:], m[:], -1.0, inv[:],
                                       AX.mult, AX.mult)
        INV = G.tile([P, T], F32, tag="bc", bufs=4)
        MI = G.tile([P, T], F32, tag="bc", bufs=4)
        nc.gpsimd.partition_broadcast(INV[:], inv[:])
        nc.gpsimd.partition_broadcast(MI[:], negmi[:])
        return INV, MI

    def c1t():
        return G.tile([P, T], F32, tag="c1", bufs=2, name=uname("c1"))

    def c2t():
        return G.tile([P, T], F32, tag="c2", bufs=2, name=uname("c2"))

    def ln_correct(psum_ap, INV, MI, ws_ap, b_ap, out_ap, relu=False):
        t1 = c1t()
        nc.vector.tensor_mul(t1[:], psum_ap, INV[:])
        t2 = c2t()
        nc.vector.scalar_tensor_tensor(t2[:], MI[:], ws_ap, t1[:],
                                       AX.mult, AX.add)
        if relu:
            nc.vector.tensor_scalar(out_ap, t2[:], b_ap, 0.0, AX.add, AX.max)
        else:
            nc.vector.tensor_scalar(out_ap, t2[:], b_ap, None, AX.add)

    def dbg_dump(name, t):
        if debug:
            dma(d[name].ap().rearrange("(k p) t -> p k t", p=P),
                t[:].bitcast(F32))

    # ============ phases 0-3 (adapters + qkv) : scoped pools ============
    with tc.tile_pool(name="WA", bufs=2) as WA, \
         tc.tile_pool(name="psA", bufs=5, space="PSUM") as psA:

        def load_w(name, kt_n, N):
            t = WA.tile([P, kt_n, N], F32R, tag="W", bufs=2,
                        padded_shape=[P, KT, D])
            dma(t[:], d[name].ap().rearrange("k p n -> p k n").bitcast(F32R))
            return t

        def proj(w_t, src, out_cb, nkt=KT):
            for m in range(KT):
                acc = psA.tile([P, T], F32, tag="mm", bufs=5)
                for k in range(nkt):
                    nc.tensor.matmul(acc[:], w_t[:, k, m * P:(m + 1) * P],
                                     src[:, k], start=(k == 0),
                                     stop=(k == nkt - 1))
                out_cb(m, acc[:])

        # ---- phase 0: input + prenorm ----
        x_sb = big()
        dma(x_sb[:],
            d["xT"].ap().rearrange("(k p) t -> p k t", p=P).bitcast(F32R))
        pw = load_vec("pw", KT)
        pb = load_vec("pb", KT)
        INVx, MIx = stats(x_sb)
        h_sb = big()
        for k in range(KT):
            t1 = c1t()
            nc.vector.tensor_mul(t1[:], x_sb[:, k], INVx[:])
            t2 = c2t()
            nc.vector.tensor_add(t2[:], t1[:], MIx[:])
            nc.vector.tensor_scalar(h_sb[:, k], t2[:],
                                    pw[:, k:k + 1], pb[:, k:k + 1],
                                    AX.mult, AX.add)
        dbg_dump("dbg_h", h_sb)
        INVh, MIh = stats(h_sb)

        # ---- phase 1: ma branch -> domain logits ----
        w1f = load_w("w1f", KT, D)
        b1f = load_vec("b1f", KT)
        ws1 = load_vec("ws1", KT)
        dq_sb = big()
        proj(w1f, h_sb,
             lambda m, acc: ln_correct(acc, INVh, MIh, ws1[:, m:m + 1],
                                       b1f[:, m:m + 1], dq_sb[:, m],
                                       relu=True))
        wlog = load_w("wlog", KT, ND)
        blog = load_vec("blog", 1, parts=ND)
        accl_t = pst.tile([ND, T], F32, tag="st", bufs=2)
        accl = accl_t[0:ND, :]
        for k in range(KT):
            nc.tensor.matmul(accl, wlog[:, k], dq_sb[:, k],
                             start=(k == 0), stop=(k == KT - 1))
        logv = G.tile([ND, T], F32, tag="logv")
        nc.vector.tensor_scalar(logv[:], accl, blog[:, 0:1], None, AX.add)
        dma(d["logT"].ap(), logv[:])

        # ---- phase 2: ta branch -> a ----
        wt1 = load_w("wt1", KT, AD)
        bt1 = load_vec("bt1", 2)
        wst1 = load_vec("wst1", 2)
        t1_sb = G.tile([P, 2, T], F32R, tag="t1sb")
        for m in range(2):
            acc = psA.tile([P, T], F32, tag="mm", bufs=5)
            for k in range(KT):
                nc.tensor.matmul(acc[:], wt1[:, k, m * P:(m + 1) * P],
                                 h_sb[:, k], start=(k == 0),
                                 stop=(k == KT - 1))
            ln_correct(acc[:], INVh, MIh, wst1[:, m:m + 1], bt1[:, m:m + 1],
                       t1_sb[:, m], relu=True)
        wt2 = load_w("wt2", 2, D)
        bt2 = load_vec("bt2", KT)
        t2_sb = big()
        proj(wt2, t1_sb,
             lambda m, acc: nc.vector.tensor_scalar(
                 t2_sb[:, m], acc, bt2[:, m:m + 1], None, AX.add),
             nkt=2)
        INVt, MIt = stats(t2_sb)
        sw2 = load_vec("sw2", KT)
        sb2 = load_vec("sb2", KT)
        a_sb = big()
        for k in range(KT):
            t1 = c1t()
            nc.vector.tensor_mul(t1[:], t2_sb[:, k], INVt[:])
            t2v = c2t()
            nc.vector.tensor_add(t2v[:], t1[:], MIt[:])
            u = c1t()
            nc.vector.tensor_scalar(u[:], t2v[:], sw2[:, k:k + 1],
                                    sb2[:, k:k + 1], AX.mult, AX.add)
            nc.vector.tensor_add(a_sb[:, k], u[:], h_sb[:, k])
        dbg_dump("dbg_a", a_sb)

        # ---- phase 3: k/v/q projections + gathers ----
        wk = load_w("wk", KT, D)
        bk = load_vec("bk", KT)
        k_own = G.tile([P, 4, H * 65], BF16, tag="BF", bufs=2)
        k_own_v = k_own[:].rearrange("p f c -> p (f c)")[:, 0:KT * T] \
            .rearrange("p (k t) -> p k t", t=T)
        proj(wk, a_sb,
             lambda m, acc: nc.vector.tensor_scalar(
                 k_own_v[:, m], acc, bk[:, m:m + 1], None, AX.add))
        dma(d["kg_in"].ap().rearrange("(k p) t -> p k t", p=P), k_own_v[:])
        nc.gpsimd.collective_compute(
            "AllGather", AX.bypass, replica_groups=GROUPS,
            ins=[d["kg_in"].ap()[:]], outs=[d["kg_out"].ap()[:]])

        wv = load_w("wv", KT, D)
        vb = WA.tile([1, D], F32R, tag="vb")
        dma(vb[:], d["vb"].ap().bitcast(F32R))
        v_own = G.tile([P, 4, H * 65], BF16, tag="BF", bufs=2)
        nc.gpsimd.memset(
            v_own[:].rearrange("p f (h c) -> p f h c", c=65)[:, :, :, 64:65],
            1.0)
        for tt in range(4):
            for c in range(2):
                acc = psA.tile([P, T], F32, tag="mm", bufs=5)
                nc.tensor.matmul(acc[:], ones_row[:],
                                 vb[:, c * T:(c + 1) * T],
                                 start=True, stop=False)
                for k in range(KT):
                    nc.tensor.matmul(acc[:], a_sb[:, k, tt * P:(tt + 1) * P],
                                     wv[:, k, c * T:(c + 1) * T],
                                     start=False, stop=(k == KT - 1))
                dst = v_own[:, tt].rearrange("p (h c) -> p h c", c=65)
                nc.vector.tensor_copy(
                    dst[:, c * 8:(c + 1) * 8, 0:64],
                    acc[:].rearrange("p (h e) -> p h e", e=64))
        dma(d["vg_in"].ap().rearrange("(f p) c -> p f c", p=P), v_own[:])
        nc.gpsimd.collective_compute(
            "AllGather", AX.bypass, replica_groups=GROUPS,
            ins=[d["vg_in"].ap()[:]], outs=[d["vg_out"].ap()[:]])

        wq = load_w("wq", KT, D)
        bq = load_vec("bq", KT)
        q_sb = G.tile([P, 4, H * 65], BF16, tag="BF", bufs=2)
        q_v = q_sb[:].rearrange("p f c -> p (f c)")[:, 0:KT * T] \
            .rearrange("p (k t) -> p k t", t=T)
        proj(wq, a_sb,
             lambda m, acc: nc.vector.tensor_scalar(
                 q_v[:, m], acc, bq[:, m:m + 1], None, AX.add))
        if debug:
            dma(d["dbg_q"].ap().rearrange("(k p) t -> p k t", p=P), q_v[:])

    # ============ phases 4-5 (attention) : scoped pools ============
    comb = big()
    with tc.tile_pool(name="KV", bufs=1) as KV, \
         tc.tile_pool(name="psT", bufs=1, space="PSUM") as psT:
        k1 = KV.tile([P, KT, L], BF16, tag="k1")
        for blk in range(4):
            dma(k1[:, :, blk * T:(blk + 1) * T],
                d["kg_out"].ap()[blk * D:(blk + 1) * D, :]
                .rearrange("(k p) t -> p k t", p=P))
        v1 = KV.tile([P, 16, H * 65], BF16, tag="v1")
        dma(v1[:], d["vg_out"].ap().rearrange("(f p) c -> p f c", p=P))
        if debug:
            dma(d["dbg_k"].ap(), d["kg_out"].ap())
            dma(d["dbg_v"].ap(), d["vg_out"].ap())
        p2 = KV.tile([P, 64], BF16, tag="p2")
        dma(p2[:], d["p2"].ap())

        for h in range(H):
            pb_ = (h % 2) * 64
            kt_ = h // 2
            hs = h * 65
            k2h = KV.tile([P, L // 2], BF16, tag="k2h", bufs=2)
            srcp = k1[pb_:pb_ + 64, kt_].rearrange("p (t two) -> p t two",
                                                   two=2)
            nc.vector.tensor_add(k2h[pb_:pb_ + 64, :], srcp[:, :, 0],
                                 srcp[:, :, 1])
            k4h = KV.tile([P, L // 4], BF16, tag="k4h", bufs=2)
            srcp2 = k2h[pb_:pb_ + 64, :].rearrange("p (t two) -> p t two",
                                                   two=2)
            nc.vector.tensor_add(k4h[pb_:pb_ + 64, :], srcp2[:, :, 0],
                                 srcp2[:, :, 1])

            # per-head pooled v (scale 2 and 4), ones cols set separately
            v2h = KV.tile([P, 8, 65], BF16, tag="v2h", bufs=2)
            nc.gpsimd.memset(v2h[:, :, 64:65], 1.0)
            for j in range(8):
                accp = psT.tile([P, 64], F32, tag="pv", bufs=3)
                nc.tensor.matmul(accp[0:64, :], p2[:, 0:64],
                                 v1[:, 2 * j, hs:hs + 64],
                                 start=True, stop=True)
                nc.tensor.matmul(accp[64:128, :], p2[:, 0:64],
                                 v1[:, 2 * j + 1, hs:hs + 64],
                                 start=True, stop=True)
                nc.vector.tensor_copy(v2h[:, j, 0:64], accp[:, :])
            v4h = KV.tile([P, 4, 65], BF16, tag="v4h", bufs=2)
            nc.gpsimd.memset(v4h[:, :, 64:65], 1.0)
            for j in range(4):
                accp = psT.tile([P, 64], F32, tag="pv", bufs=3)
                nc.tensor.matmul(accp[0:64, :], p2[:, 0:64],
                                 v2h[:, 2 * j, 0:64], start=True, stop=True)
                nc.tensor.matmul(accp[64:128, :], p2[:, 0:64],
                                 v2h[:, 2 * j + 1, 0:64],
                                 start=True, stop=True)
                nc.vector.tensor_copy(v4h[:, j, 0:64], accp[:, :])

            qh = q_sb[:].rearrange("p f c -> p (f c)")[:, 0:KT * T] \
                .rearrange("p (k t) -> p k t", t=T)[pb_:pb_ + 64, kt_]
            cslice = comb[pb_:pb_ + 64, kt_]
            for si, s in enumerate(SCALES):
                nk = L // s // P
                pv = psT.tile([65, T], F32, tag="pv", bufs=3)
                for g in range((nk + 1) // 2):
                    gw = min(2, nk - g * 2)
                    scp = psT.tile([P, 2, T], F32, tag="scg", bufs=1)
                    for i in range(gw):
                        ti = g * 2 + i
                        if s == 1:
                            kl = k1[pb_:pb_ + 64, kt_, ti * P:(ti + 1) * P]
                        elif s == 2:
                            kl = k2h[pb_:pb_ + 64, ti * P:(ti + 1) * P]
                        else:
                            kl = k4h[pb_:pb_ + 64, ti * P:(ti + 1) * P]
                        nc.tensor.matmul(scp[:, i], kl, qh,
                                         start=True, stop=True)
                    ex = KV.tile([P, 2, T], BF16, tag="ex", bufs=2)
                    nc.scalar.activation(
                        ex[:, 0:gw].rearrange("p g t -> p (g t)"),
                        scp[:, 0:gw].rearrange("p g t -> p (g t)"),
                        AF.Exp, scale=1.0 / s)
                    if debug and h == 0 and s == 1 and g == 0:
                        dma(d["dbg_ex"].ap(), ex[:, 0])
                    for i in range(gw):
                        ti = g * 2 + i
                        if s == 1:
                            vl = v1[:, ti, hs:hs + 65]
                        elif s == 2:
                            vl = v2h[:, ti, 0:65]
                        else:
                            vl = v4h[:, ti, 0:65]
                        nc.tensor.matmul(pv[:], vl, ex[:, i],
                                         start=(ti == 0), stop=(ti == nk - 1))
                rcp = KV.tile([65, T], F32, tag="rcp", bufs=2)
                nc.vector.tensor_copy(rcp[64:65, :], pv[64:65, :])
                den0 = KV.tile([1, T], F32, tag="den0", bufs=1)
                dma(den0[:], rcp[64:65, :])
                rec0 = KV.tile([1, T], F32, tag="rec0", bufs=2)
                nc.vector.reciprocal_approx_fast(rec0[:], den0[:])
                if debug and h == 0 and s == 1:
                    pvd = KV.tile([65, T], F32, tag="cmb", bufs=1)
                    nc.vector.tensor_copy(pvd[:], pv[:])
                    dma(d["dbg_pv"].ap(), pvd[:])
                    dma(d["dbg_rec"].ap(), rec0[:])
                REC = KV.tile([128, T], F32, tag="rec_bc", bufs=2)
                nc.gpsimd.partition_broadcast(REC[:], rec0[:])
                if si == 0:
                    nc.vector.scalar_tensor_tensor(
                        cslice, pv[0:64, :], c_comb[si],
                        REC[pb_:pb_ + 64, :], AX.mult, AX.mult)
                else:
                    tmp = KV.tile([128, T], F32, tag="cmb", bufs=1)
                    nc.vector.scalar_tensor_tensor(
                        tmp[pb_:pb_ + 64, :], pv[0:64, :], c_comb[si],
                        REC[pb_:pb_ + 64, :], AX.mult, AX.mult)
                    nc.vector.tensor_add(cslice, cslice,
                                         tmp[pb_:pb_ + 64, :])
    dbg_dump("dbg_comb", comb)

    # ============ phases 6-9 : scoped pools ============
    with tc.tile_pool(name="WB", bufs=2) as WB, \
         tc.tile_pool(name="psB", bufs=6, space="PSUM") as psB:

        def load_wb(name, kt_n, N):
            t = WB.tile([P, kt_n, N], F32R, tag="W", bufs=1,
                        padded_shape=[P, KT, D])
            dma(t[:], d[name].ap().rearrange("k p n -> p k n").bitcast(F32R))
            return t

        def projB(w_t, src, out_cb, nkt=KT):
            for m in range(KT):
                acc = psB.tile([P, T], F32, tag="mm", bufs=6)
                for k in range(nkt):
                    nc.tensor.matmul(acc[:], w_t[:, k, m * P:(m + 1) * P],
                                     src[:, k], start=(k == 0),
                                     stop=(k == nkt - 1))
                out_cb(m, acc[:])

        # ---- phase 6: out-proj + attn LN ----
        wo = load_wb("wo", KT, D)
        bo = load_vec("bo", KT)
        z_sb = big()

        def z_cb(m, acc):
            t1 = c1t()
            nc.vector.tensor_scalar(t1[:], acc, bo[:, m:m + 1], None, AX.add)
            nc.vector.tensor_add(z_sb[:, m], t1[:], a_sb[:, m])

        projB(wo, comb, z_cb)
        INVz, MIz = stats(z_sb)
        attw = load_vec("attw", KT)
        attb = load_vec("attb", KT)
        attn = big()
        for k in range(KT):
            t1 = c1t()
            nc.vector.tensor_mul(t1[:], z_sb[:, k], INVz[:])
            t2 = c2t()
            nc.vector.tensor_add(t2[:], t1[:], MIz[:])
            nc.vector.tensor_scalar(attn[:, k], t2[:],
                                    attw[:, k:k + 1], attb[:, k:k + 1],
                                    AX.mult, AX.add)
        dbg_dump("dbg_attn", attn)
        INVa, MIa = stats(attn)

        # ---- phase 7: router ----
        wr = load_wb("wr", KT, NE)
        brf = load_vec("brf", 1, parts=NE)
        wsr = load_vec("wsr", 1, parts=NE)
        accr = pst.tile([ND, T], F32, tag="st", bufs=2)
        for k in range(KT):
            nc.tensor.matmul(accr[0:NE, :], wr[:, k], attn[:, k],
                             start=(k == 0), stop=(k == KT - 1))
        rrow = lambda dt=F32: G.tile([NE, T], dt, tag="rrow", bufs=3,
                                     name=uname("rrow"))
        rl1 = rrow()
        nc.vector.tensor_mul(rl1[:], accr[0:NE, :], INVa[0:NE, :])
        rl2 = rrow()
        nc.vector.scalar_tensor_tensor(rl2[:], MIa[0:NE, :], wsr[:, 0:1],
                                       rl1[:], AX.mult, AX.add)
        nc.vector.tensor_scalar(rl2[:], rl2[:], brf[:, 0:1], None, AX.add)
        rexp = rrow(F32R)
        nc.scalar.activation(rexp[:], rl2[:], AF.Exp)
        racc = pst.tile([1, T], F32, tag="st", bufs=2)
        nc.tensor.matmul(racc[:], ones_r[0:NE, :], rexp[:],
                         start=True, stop=True)
        rdc = G.tile([1, T], F32, tag="srow", bufs=4)
        nc.vector.tensor_copy(rdc[:], racc[:])
        rden = G.tile([1, T], F32, tag="srow", bufs=4)
        nc.vector.reciprocal_approx_fast(rden[:], rdc[:])
        RDEN = rrow()
        nc.gpsimd.partition_broadcast(RDEN[:], rden[:])
        rwp = rrow()
        nc.vector.tensor_mul(rwp[:], rexp[:], RDEN[:])
        rmask = rrow()
        nc.vector.tensor_scalar(rmask[:], rwp[:], thr, None, AX.is_gt)
        rwm = rrow(F32R)
        nc.vector.tensor_mul(rwm[:], rwp[:], rmask[:])
        racc2 = pst.tile([1, T], F32, tag="st", bufs=2)
        nc.tensor.matmul(racc2[:], ones_r[0:NE, :], rwm[:],
                         start=True, stop=True)
        rden2 = G.tile([1, T], F32, tag="srow", bufs=4)
        nc.vector.tensor_scalar(rden2[:], racc2[:], 1e-6, None, AX.add)
        nc.vector.reciprocal_approx_fast(rden2[:], rden2[:])
        RDEN2 = rrow()
        nc.gpsimd.partition_broadcast(RDEN2[:], rden2[:])
        rwf = G.tile([NE, T], F32, tag="rwf")
        nc.vector.tensor_mul(rwf[:], rwm[:], RDEN2[:])
        if debug:
            dma(d["dbg_rw"].ap(), rwf[:])

        # ---- phase 8: MoE ----
        acc_sb = big()
        abf = G.tile([P, 4, H * 65], BF16, tag="BF", bufs=2)
        abf_v = abf[:].rearrange("p f c -> p (f c)")[:, 0:KT * T] \
            .rearrange("p (k t) -> p k t", t=T)
        for k in range(KT):
            nc.vector.tensor_copy(abf_v[:, k], attn[:, k])
        with tc.tile_pool(name="EW", bufs=1) as EW:
            for e in range(NE):
                rwe0 = G.tile([1, T], F32, tag="rwe0", bufs=1,
                              name=uname("rwe0"))
                dma(rwe0[:], rwf[e:e + 1, :])
                RWe = G.tile([P, T], F32, tag="rwe", bufs=1,
                             name=uname("rwe"))
                nc.gpsimd.partition_broadcast(RWe[:], rwe0[:])
                b1 = EW.tile([P, 32], F32, tag="eb1", bufs=2,
                             name=uname("eb1"))
                dma(b1[:], d["eb1"].ap()[e])
                b2 = EW.tile([P, KT], F32, tag="eb2", bufs=2,
                             name=uname("eb2"))
                dma(b2[:], d["eb2"].ap()[e])

                h1 = EW.tile([P, 32, T], BF16, tag="h1", bufs=1,
                             name=uname("h1"))
                for mc in range(4):
                    w1r = EW.tile([P, KT, 1024], BF16, tag="ewr", bufs=2,
                                  name=uname("w1r"))
                    dma(w1r[:], d["ew1"].ap()[e]
                        .rearrange("k p n -> p k n")
                        [:, :, mc * 1024:(mc + 1) * 1024])
                    for ml in range(8):
                        m = mc * 8 + ml
                        acc1 = psB.tile([P, T], F32, tag="mm", bufs=6,
                                        name=uname("acc1"))
                        for k in range(KT):
                            nc.tensor.matmul(
                                acc1[:], w1r[:, k, ml * P:(ml + 1) * P],
                                abf_v[:, k], start=(k == 0),
                                stop=(k == KT - 1))
                        nc.scalar.activation(h1[:, m], acc1[:], AF.Relu,
                                             bias=b1[:, m:m + 1])
                for mc2 in range(2):
                    accs = [psB.tile([P, T], F32, tag="mm", bufs=6,
                                     name=uname("acc2"))
                            for _ in range(4)]
                    for kc in range(4):
                        w2r = EW.tile([P, KT, 512], BF16, tag="ewr", bufs=2,
                                      name=uname("w2r"))
                        dma(w2r[:], d["ew2"].ap()[e]
                            [kc * 8:(kc + 1) * 8]
                            .rearrange("k p n -> p k n")
                            [:, :, mc2 * 512:(mc2 + 1) * 512])
                        for ml in range(4):
                            for kl in range(8):
                                k = kc * 8 + kl
                                nc.tensor.matmul(
                                    accs[ml][:],
                                    w2r[:, kl, ml * P:(ml + 1) * P],
                                    h1[:, k],
                                    start=(k == 0), stop=(k == 31))
                    for ml in range(4):
                        m = mc2 * 4 + ml
                        u = c1t()
                        nc.vector.tensor_scalar(u[:], accs[ml][:],
                                                b2[:, m:m + 1], None, AX.add)
                        w_ = c2t()
                        nc.vector.tensor_mul(w_[:], u[:], RWe[:])
                        if e == 0:
                            nc.vector.tensor_add(acc_sb[:, m], w_[:],
                                                 attn[:, m])
                        else:
                            nc.vector.tensor_add(acc_sb[:, m],
                                                 acc_sb[:, m], w_[:])

        # ---- phase 9: final LN + residual ----
        INVu, MIu = stats(acc_sb)
        moew = load_vec("moew", KT)
        moeb = load_vec("moeb", KT)
        x2_sb = G.tile([P, KT, T], F32, tag="ACT16", bufs=3)
        dma(x2_sb[:], d["xT"].ap().rearrange("(k p) t -> p k t", p=P))
        out_sb = G.tile([P, KT, T], F32, tag="ACT16", bufs=3)
        for k in range(KT):
            t1 = c1t()
            nc.vector.tensor_mul(t1[:], acc_sb[:, k], INVu[:])
            t2 = c2t()
            nc.vector.tensor_add(t2[:], t1[:], MIu[:])
            t3 = c1t()
            nc.vector.tensor_scalar(t3[:], t2[:], moew[:, k:k + 1],
                                    moeb[:, k:k + 1], AX.mult, AX.add)
            nc.vector.tensor_add(out_sb[:, k], t3[:], x2_sb[:, k])
            dma(d["outT"].ap().rearrange("(k p) t -> p k t", p=P)[:, k],
                out_sb[:, k])


# --------------------------------------------------------------------------
# host side
# --------------------------------------------------------------------------

def _fold_params(p):
    g = {k: np.asarray(v, dtype=np.float32) for k, v in p.items()}
    inv = 1.0 / np.sqrt(HD)
    sw = np.exp(g["scale_weights"] - g["scale_weights"].max())
    sw = sw / sw.sum()
    c_comb = [float(sw[i] / s) for i, s in enumerate(SCALES)]
    thr = float(np.ravel(g["thr"])[0])
    temp = float(np.ravel(g["temperature"])[0])
    sca = float(np.ravel(g["adapter_scale"])[0])

    def tile_w(W):
        return np.ascontiguousarray(W.reshape(W.shape[0] // P, P, W.shape[1]))

    def vec_t(v, n):
        return np.ascontiguousarray(v.reshape(n, P).T)

    w1f = g["ma_ln_w"][:, None] * g["ma_w1"]
    b1f = g["ma_b1"] + g["ma_ln_b"] @ g["ma_w1"]
    wlog = (g["ma_w2"] @ g["protos"].T) / temp
    blog = (g["ma_b2"] @ g["protos"].T) / temp
    wt1 = g["ta_ln1_w"][:, None] * g["ta_w1"]
    bt1 = g["ta_b1"] + g["ta_ln1_b"] @ g["ta_w1"]
    wr = g["r_ln_w"][:, None] * g["r_w"]
    brf = g["r_b"] + g["r_ln_b"] @ g["r_w"]

    p2 = np.zeros((P, 64), dtype=np.float32)
    for j in range(P):
        p2[j, j // 2] = 1.0
    bf = ml_dtypes.bfloat16

    weights = {
        "pw": vec_t(g["prenorm_w"], KT), "pb": vec_t(g["prenorm_b"], KT),
        "w1f": tile_w(w1f), "b1f": vec_t(b1f, KT),
        "ws1": vec_t(w1f.sum(axis=0), KT),
        "wlog": tile_w(wlog), "blog": blog.reshape(ND, 1),
        "wt1": tile_w(wt1), "bt1": vec_t(bt1, 2),
        "wst1": vec_t(wt1.sum(axis=0), 2),
        "wt2": tile_w(g["ta_w2"]), "bt2": vec_t(g["ta_b2"], KT),
        "sw2": vec_t(sca * g["ta_ln2_w"], KT),
        "sb2": vec_t(sca * g["ta_ln2_b"], KT),
        "wq": tile_w(inv * g["q_w"]), "bq": vec_t(inv * g["q_b"], KT),
        "wk": tile_w(g["k_w"]), "bk": vec_t(g["k_b"], KT),
        "wv": tile_w(g["v_w"]), "vb": g["v_b"].reshape(1, D),
        "wo": tile_w(g["o_w"]), "bo": vec_t(g["o_b"], KT),
        "p2": p2.astype(bf),
        "attw": vec_t(g["attn_ln_w"], KT), "attb": vec_t(g["attn_ln_b"], KT),
        "wr": tile_w(wr), "brf": brf.reshape(NE, 1),
        "wsr": wr.sum(axis=0).reshape(NE, 1),
        "ew1": np.ascontiguousarray(
            g["e_w1"].reshape(NE, KT, P, 4 * D)).astype(bf),
        "eb1": np.ascontiguousarray(
            g["e_b1"].reshape(NE, 32, P).transpose(0, 2, 1)),
        "ew2": np.ascontiguousarray(
            g["e_w2"].reshape(NE, 32, P, D)).astype(bf),
        "eb2": np.ascontiguousarray(
            g["e_b2"].reshape(NE, KT, P).transpose(0, 2, 1)),
        "moew": vec_t(g["moe_ln_w"], KT), "moeb": vec_t(g["moe_ln_b"], KT),
        "ones_c": np.ones((P, 1), dtype=np.float32),
        "ones_rw": np.ones((1, P), dtype=np.float32),
    }
    scalars = {"c_comb": c_comb, "thr": thr}
    key = (tuple(c_comb), thr, bool(os.environ.get("BASSK_DEBUG")))
    return weights, scalars, key


def get_program(params):
    weights, scalars, key = _fold_params(params)
    if key not in _COMPILED:
        _COMPILED[key] = _build_program(
            scalars, debug=bool(os.environ.get("BASSK_DEBUG")))
    return _COMPILED[key], weights


def run(x, params):
    from concourse.bass_utils import run_bass_kernel_spmd

    x = np.asarray(x, dtype=np.float32)
    nc, weights = get_program(params)
    xf = x.reshape(B * L, D)
    in_maps = []
    for c in range(NCORES):
        m = dict(weights)
        m["xT"] = np.ascontiguousarray(xf[c * T:(c + 1) * T].T)
        in_maps.append(m)
    res = run_bass_kernel_spmd(nc, in_maps, list(range(NCORES)))
    return res




_RUNNER = {}


def make_runner(params):
    """Build (once) a persistent jitted SPMD callable with device-resident
    weights. Returns run_fn(x_np) -> (out, probs)."""
    import jax
    from jax.sharding import Mesh, PartitionSpec
    from jax.experimental.shard_map import shard_map
    import jax.numpy as jnp
    from concourse import bass2jax
    from concourse.bass2jax import _bass_exec_p, partition_id_tensor

    weights, scalars, key = _fold_params(params)
    if key in _RUNNER:
        return _RUNNER[key]
    if key not in _COMPILED:
        _COMPILED[key] = _build_program(
            scalars, debug=bool(os.environ.get("BASSK_DEBUG")))
    nc = _COMPILED[key]
    bass2jax.install_neuronx_cc_hook()

    partition_name = (nc.partition_id_tensor.name
                      if nc.partition_id_tensor else None)
    in_names, out_names, out_avals, zero_shapes = [], [], [], []
    for alloc in nc.m.functions[0].allocations:
        if not isinstance(alloc, mybir.MemoryLocationSet):
            continue
        name = alloc.memorylocations[0].name
        if alloc.kind == "ExternalInput":
            if name != partition_name:
                in_names.append(name)
        elif alloc.kind == "ExternalOutput":
            shape = tuple(alloc.tensor_shape)
            dtype = mybir.dt.np(alloc.dtype)
            out_names.append(name)
            out_avals.append(jax.core.ShapedArray(shape, dtype))
            zero_shapes.append((shape, dtype))
    n_params = len(in_names)
    all_names = list(in_names) + list(out_names)
    if partition_name is not None:
        all_names.append(partition_name)

    def _body(*args):
        operands = list(args)
        if partition_name is not None:
            operands.append(partition_id_tensor())
        outs = _bass_exec_p.bind(
            *operands,
            out_avals=tuple(out_avals),
            in_names=tuple(all_names),
            out_names=tuple(out_names),
            lowering_input_output_aliases=(),
            sim_require_finite=True,
            sim_require_nnan=True,
            nc=nc,
        )
        return tuple(outs)

    devices = jax.devices()[:NCORES]
    mesh = Mesh(np.asarray(devices), ("core",))
    in_specs = (PartitionSpec("core"),) * (n_params + len(out_names))
    out_specs = (PartitionSpec("core"),) * len(out_names)
    sharded = jax.jit(shard_map(_body, mesh=mesh, in_specs=in_specs,
                                out_specs=out_specs, check_rep=False),
                      keep_unused=True)

    zeros_dev = [
        jax.device_put(np.zeros((NCORES * s[0], *s[1:]), dt),
                       jax.sharding.NamedSharding(mesh, PartitionSpec("core")))
        for s, dt in zero_shapes]

    # device-resident weights (identical per core -> concat replicas)
    w_dev = {}
    for name in in_names:
        if name == "xT":
            continue
        w = weights[name]
        w_dev[name] = jax.device_put(
            np.concatenate([w] * NCORES, axis=0),
            jax.sharding.NamedSharding(mesh, PartitionSpec("core")))

    def run_fn(x_np, fetch=True):
        xf = np.asarray(x_np, dtype=np.float32).reshape(B * L, D)
        xT = np.concatenate(
            [np.ascontiguousarray(xf[c * T:(c + 1) * T].T)
             for c in range(NCORES)], axis=0)
        args = [xT if n == "xT" else w_dev[n] for n in in_names]
        outs = sharded(*args, *zeros_dev)
        if not fetch:
            jax.block_until_ready(outs)
            return None
        res = {n: np.asarray(outs[i]) for i, n in enumerate(out_names)}
        out = np.empty((B * L, D), dtype=np.float32)
        logits = np.empty((B * L, ND), dtype=np.float32)
        for c in range(NCORES):
            out[c * T:(c + 1) * T] = res["outT"][c * D:(c + 1) * D].T
            logits[c * T:(c + 1) * T] = res["logT"][c * ND:(c + 1) * ND].T
        e = np.exp(logits - logits.max(axis=-1, keepdims=True))
        probs = e / e.sum(axis=-1, keepdims=True)
        return (out.reshape(B, L, D), probs.reshape(B, L, ND))

    _RUNNER[key] = run_fn
    return run_fn


def kernel(x, params):
    run_fn = make_runner(params)
    out, probs = run_fn(x)
    return out, probs.astype(np.float32)
